# revision 1
# baseline (speedup 1.0000x reference)
import numpy as np

IN_CAPS = 1152
OUT_CAPS = 10
IN_DIM = 8
OUT_DIM = 16
JD = OUT_CAPS * OUT_DIM  # 160
BATCH = 512
N_CORES = 8
# 2D sharding: 4 batch-shards x 2 i-shards -> M=128 matmuls, full-width evac
NB = 4                 # batch shards
BC = BATCH // NB       # 128 samples per core
IH = IN_CAPS // 2      # 576 i-caps per core
G = 24                 # i-caps per group
NG = IH // G           # 24 groups
IPB = 3                # i per psum bank tile (3*160=480 fp32 <= 512)

_cached = {}


def _install_ntff_hook():
    try:
        import sys, types, ctypes, contextlib

        if "antenv.axon_hooks" not in sys.modules:
            mod = types.ModuleType("antenv.axon_hooks")
            holder = {}
            mod.set_axon_ntff_profile_hook = lambda h: holder.__setitem__("h", h)
            mod.get_axon_ntff_profile_hook = lambda: holder.get("h")
            sys.modules["antenv.axon_hooks"] = mod
            try:
                import antenv

                antenv.axon_hooks = mod
            except Exception:
                pass
            lib = ctypes.CDLL("/opt/axon/libaxon_pjrt.so")
            if hasattr(lib, "axon_start_nrt_profile"):
                lib.axon_start_nrt_profile.argtypes = [
                    ctypes.POINTER(ctypes.c_int64),
                    ctypes.c_size_t,
                ]
                lib.axon_start_nrt_profile.restype = ctypes.c_int64
                lib.axon_stop_nrt_profile.argtypes = [ctypes.c_char_p]
                lib.axon_stop_nrt_profile.restype = ctypes.c_int64

                @contextlib.contextmanager
                def _hook(output_dir, device_ids):
                    import jax

                    jax.devices()
                    if device_ids:
                        ids = (ctypes.c_int64 * len(device_ids))(*device_ids)
                        rc = lib.axon_start_nrt_profile(ids, len(device_ids))
                    else:
                        rc = lib.axon_start_nrt_profile(None, 0)
                    if rc != 0:
                        raise RuntimeError(f"axon_start_nrt_profile rc={rc}")
                    try:
                        yield
                    finally:
                        lib.axon_stop_nrt_profile(str(output_dir).encode())

                mod.set_axon_ntff_profile_hook(_hook)
        import concourse.bass_utils as bu

        bu.upload_artifacts = lambda tmpdir: tmpdir
    except Exception:
        pass


def _build_nc():
    import concourse.bass as bass
    import concourse.tile as tile
    from concourse import bacc, mybir

    nc = bacc.Bacc("TRN2", target_bir_lowering=False, debug=False)
    f32 = mybir.dt.float32
    bf16 = mybir.dt.bfloat16

    # host-prearranged inputs (bf16):
    # xt: [1152, 8, 64]   = x[b,i,e] -> [i, e, b]
    # wt: [1152, 8, 160]  = W[i,j,d,e] -> [i, e, j*16+d]
    xt_d = nc.dram_tensor("xt", [NG, IN_DIM, G * BC], bf16, kind="ExternalInput")
    wt_d = nc.dram_tensor("wt", [NG, IN_DIM, G * JD], bf16, kind="ExternalInput")
    # u: [g, b, i_local*160+jd] bf16 (contiguous stores, host unpacks)
    u_d = nc.dram_tensor("u", [NG, BC, G * JD], bf16, kind="ExternalOutput")

    with tile.TileContext(nc) as tc:
        with (
            tc.tile_pool(name="xp", bufs=4) as xp,
            tc.tile_pool(name="wp", bufs=4) as wp,
            tc.tile_pool(name="sp", bufs=6) as sp,
            tc.tile_pool(name="pp", bufs=8, space="PSUM") as pp,
        ):
            for g in range(NG):
                i0 = g * G
                xt_t = xp.tile([IN_DIM, G * BC], bf16)
                nc.sync.dma_start(xt_t[:], xt_d[g])
                wt_t = wp.tile([IN_DIM, G * JD], bf16)
                nc.sync.dma_start(wt_t[:], wt_d[g])
                half = (G // IPB) // 4  # 2 psum tiles per quarter
                hw = half * IPB * JD    # 960
                for hh in range(4):
                    st_t = sp.tile([BC, hw], bf16)
                    for kk in range(half):
                        k = hh * half + kk
                        ps = pp.tile([BC, IPB * JD], f32)
                        for m in range(IPB):
                            ii = k * IPB + m
                            nc.tensor.matmul(
                                ps[:, m * JD : (m + 1) * JD],
                                xt_t[:, ii * BC : (ii + 1) * BC],
                                wt_t[:, ii * JD : (ii + 1) * JD],
                                start=True,
                                stop=True,
                            )
                        o = kk * IPB * JD
                        if k % 2 == 0:
                            nc.vector.tensor_copy(st_t[:, o : o + IPB * JD], ps[:])
                        else:
                            nc.scalar.copy(st_t[:, o : o + IPB * JD], ps[:])
                    nc.sync.dma_start(u_d[g][:, hh * hw : (hh + 1) * hw], st_t[:])
    nc.finalize()
    return nc


def _routing(u):
    B = u.shape[0]
    b = np.zeros((B, IN_CAPS, OUT_CAPS), dtype=np.float32)
    v = None
    for it in range(3):
        m = b.max(axis=2, keepdims=True)
        e = np.exp(b - m)
        c = e / e.sum(axis=2, keepdims=True)
        s = np.einsum("bij,bijd->bjd", c, u, optimize=True)
        mag_sq = np.sum(s * s, axis=-1, keepdims=True)
        mag = np.sqrt(mag_sq + 1e-8)
        v = (mag_sq / (1.0 + mag_sq)) * (s / mag)
        if it != 2:
            b = b + np.einsum("bijd,bjd->bij", u, v, optimize=True)
    return v.astype(np.float32)


def _u_host(x, W):
    return np.einsum("ijde,bie->bijd", W, x, optimize=True).astype(np.float32)


def kernel(x, W):
    import ml_dtypes

    x = np.asarray(x, dtype=np.float32)
    W = np.asarray(W, dtype=np.float32)
    wtf = np.ascontiguousarray(
        W.reshape(IN_CAPS, JD, IN_DIM).transpose(0, 2, 1)
    ).astype(ml_dtypes.bfloat16)
    try:
        from concourse.bass_utils import run_bass_kernel_spmd

        _install_ntff_hook()
        if "nc" not in _cached:
            _cached["nc"] = _build_nc()
        nc = _cached["nc"]
        wqs = []
        for h in range(2):
            wh = wtf[h * IH : (h + 1) * IH]  # [576, 8, 160]
            wq = wh.reshape(NG, G, IN_DIM, JD).transpose(0, 2, 1, 3)
            wqs.append(np.ascontiguousarray(wq.reshape(NG, IN_DIM, G * JD)))
        in_maps = []
        for c in range(N_CORES):
            q, h = divmod(c, 2)
            xs = x[q * BC : (q + 1) * BC, h * IH : (h + 1) * IH]  # [128, 576, 8]
            xi = xs.transpose(1, 2, 0).reshape(NG, G, IN_DIM, BC)
            xq = np.ascontiguousarray(
                xi.transpose(0, 2, 1, 3).reshape(NG, IN_DIM, G * BC)
            ).astype(ml_dtypes.bfloat16)
            in_maps.append({"xt": xq, "wt": wqs[h]})
        try:
            res = run_bass_kernel_spmd(
                nc, in_maps, core_ids=list(range(N_CORES)), trace=True
            )
        except Exception:
            import traceback

            traceback.print_exc()
            res = run_bass_kernel_spmd(nc, in_maps, core_ids=list(range(N_CORES)))
        us = []
        for c in range(N_CORES):
            uc = np.asarray(res.results[c]["u"], dtype=np.float32)
            uc = uc.reshape(NG, BC, G, JD).transpose(1, 0, 2, 3)
            us.append(uc.reshape(BC, IH, OUT_CAPS, OUT_DIM))
        u = np.concatenate(
            [
                np.concatenate([us[2 * q], us[2 * q + 1]], axis=1)
                for q in range(NB)
            ],
            axis=0,
        )
        _cached["exec_time_ns"] = getattr(res, "exec_time_ns", None)
    except Exception:
        import traceback

        traceback.print_exc()
        u = _u_host(x, W)
    return _routing(u)



# revision 4
# speedup vs baseline: 1.7973x; 1.7973x over previous
import numpy as np

IN_CAPS = 1152
OUT_CAPS = 10
IN_DIM = 8
OUT_DIM = 16
JD = OUT_CAPS * OUT_DIM  # 160
BATCH = 512
N_CORES = 8
# 4 batch-quarters x 2 i-halves; per core: 128 batches, 576 input caps
NB = 4
BC = BATCH // NB         # 128
IH = IN_CAPS // 2        # 576
IPG = 3                  # i-caps per row-tile per superchunk
NG = IH // (4 * IPG)     # 48 superchunks
FW = IPG * JD            # 480 moving-operand cols per matmul
ROWS = IPG * IN_DIM      # 24 used contraction rows per 32-row tile
USCALE = 64.0            # u is computed scaled by 64 to keep fp8 out of subnormals

_cached = {}


def _install_ntff_hook():
    try:
        import sys, types, ctypes, contextlib

        if "antenv.axon_hooks" not in sys.modules:
            mod = types.ModuleType("antenv.axon_hooks")
            holder = {}
            mod.set_axon_ntff_profile_hook = lambda h: holder.__setitem__("h", h)
            mod.get_axon_ntff_profile_hook = lambda: holder.get("h")
            sys.modules["antenv.axon_hooks"] = mod
            try:
                import antenv

                antenv.axon_hooks = mod
            except Exception:
                pass
            lib = ctypes.CDLL("/opt/axon/libaxon_pjrt.so")
            if hasattr(lib, "axon_start_nrt_profile"):
                lib.axon_start_nrt_profile.argtypes = [
                    ctypes.POINTER(ctypes.c_int64),
                    ctypes.c_size_t,
                ]
                lib.axon_start_nrt_profile.restype = ctypes.c_int64
                lib.axon_stop_nrt_profile.argtypes = [ctypes.c_char_p]
                lib.axon_stop_nrt_profile.restype = ctypes.c_int64

                @contextlib.contextmanager
                def _hook(output_dir, device_ids):
                    import jax

                    jax.devices()
                    if device_ids:
                        ids = (ctypes.c_int64 * len(device_ids))(*device_ids)
                        rc = lib.axon_start_nrt_profile(ids, len(device_ids))
                    else:
                        rc = lib.axon_start_nrt_profile(None, 0)
                    if rc != 0:
                        raise RuntimeError(f"axon_start_nrt_profile rc={rc}")
                    try:
                        yield
                    finally:
                        lib.axon_stop_nrt_profile(str(output_dir).encode())

                mod.set_axon_ntff_profile_hook(_hook)
        import concourse.bass_utils as bu

        bu.upload_artifacts = lambda tmpdir: tmpdir
    except Exception:
        pass


def _build_nc():
    import concourse.bass as bass
    import concourse.tile as tile
    from concourse import bacc, mybir

    nc = bacc.Bacc("TRN2", target_bir_lowering=False, debug=False)
    f32 = mybir.dt.float32
    bf16 = mybir.dt.bfloat16
    f8 = mybir.dt.float8e4

    # inputs (host pre-arranged; partition p = r*32 + i3*8 + e, i3<3; rows 24..31
    # of each 32-block are zero-padded)
    xt_d = nc.dram_tensor("xt", [128, NG, BC], bf16, kind="ExternalInput")
    # wp: block-diag fp8 weights (scaled x64): [p, g, k*160+jd]
    wp_d = nc.dram_tensor("wp", [128, NG, FW], f8, kind="ExternalInput")
    # wd: dense bf16 weights for the exact S0 pass: [p, g, jd]
    wd_d = nc.dram_tensor("wd", [128, NG, JD], bf16, kind="ExternalInput")
    # outputs
    u8_d = nc.dram_tensor("u8", [NG, BC, 4 * FW], f8, kind="ExternalOutput")
    s0_d = nc.dram_tensor("s0", [BC, JD], f32, kind="ExternalOutput")

    H = NG // 2
    with tile.TileContext(nc) as tc:
        with (
            tc.tile_pool(name="cp", bufs=1) as cp,
            tc.tile_pool(name="obp", bufs=4) as obp,
            tc.tile_pool(name="pp", bufs=3, space="PSUM") as pp,
            tc.tile_pool(name="sp", bufs=1, space="PSUM") as sp,
        ):
            xt = cp.tile([128, NG, BC], bf16)
            wp = cp.tile([128, NG, FW], f8)
            wd = cp.tile([128, NG, JD], bf16)
            nc.sync.dma_start(xt[:, :H], xt_d[:, :H])
            nc.sync.dma_start(wp[:, :H], wp_d[:, :H])
            nc.sync.dma_start(xt[:, H:], xt_d[:, H:])
            nc.sync.dma_start(wp[:, H:], wp_d[:, H:])
            nc.sync.dma_start(wd[:], wd_d[:])

            for g in range(NG):
                dtA = pp.tile([128, 2, 512], f32, name="dt")
                dtB = pp.tile([128, 2, 512], f32, name="dt")
                for r in range(4):
                    dt = dtA if r < 2 else dtB
                    nc.tensor.matmul(
                        dt[:, r % 2, 0:FW],
                        xt[32 * r : 32 * r + ROWS, g, :],
                        wp[32 * r : 32 * r + ROWS, g, :],
                        start=True,
                        stop=True,
                        tile_position=(32 * r, 0),
                    )
                ob = obp.tile([128, 4, FW], f8, name="ob")
                nc.vector.tensor_copy(ob[:, 0:2, :], dtA[:, :, 0:FW])
                nc.scalar.copy(ob[:, 2:4, :], dtB[:, :, 0:FW])
                nc.sync.dma_start(u8_d[g], ob[:])

            # exact S0 = sum_i u (fp32 psum accumulation, full 128x128 mode)
            s0t = sp.tile([128, 512], f32)
            for g in range(NG):
                nc.tensor.matmul(
                    s0t[:, 0:JD],
                    xt[:, g, :],
                    wd[:, g, :],
                    start=(g == 0),
                    stop=(g == NG - 1),
                )
            s0sb = cp.tile([128, JD], f32)
            nc.vector.tensor_copy(s0sb[:], s0t[:, 0:JD])
            nc.sync.dma_start(s0_d[:], s0sb[:])
    nc.finalize()
    return nc


def _routing_s0(u, S0):
    # s = S0/10 + sum_i (c - 1/10) u  -- S0 exact, u may be quantized
    b = np.zeros(u.shape[:3], dtype=np.float32)
    v = None
    for it in range(3):
        m = b.max(axis=2, keepdims=True)
        e = np.exp(b - m)
        c = e / e.sum(axis=2, keepdims=True)
        s = 0.1 * S0 + np.einsum("bij,bijd->bjd", c - 0.1, u, optimize=True)
        mag_sq = np.sum(s * s, axis=-1, keepdims=True)
        mag = np.sqrt(mag_sq + 1e-8)
        v = (mag_sq / (1.0 + mag_sq)) * (s / mag)
        if it != 2:
            b = b + np.einsum("bijd,bjd->bij", u, v, optimize=True)
    return v.astype(np.float32)


def _routing(u):
    b = np.zeros((u.shape[0], IN_CAPS, OUT_CAPS), dtype=np.float32)
    v = None
    for it in range(3):
        m = b.max(axis=2, keepdims=True)
        e = np.exp(b - m)
        c = e / e.sum(axis=2, keepdims=True)
        s = np.einsum("bij,bijd->bjd", c, u, optimize=True)
        mag_sq = np.sum(s * s, axis=-1, keepdims=True)
        mag = np.sqrt(mag_sq + 1e-8)
        v = (mag_sq / (1.0 + mag_sq)) * (s / mag)
        if it != 2:
            b = b + np.einsum("bijd,bjd->bij", u, v, optimize=True)
    return v.astype(np.float32)


def _u_host(x, W):
    return np.einsum("ijde,bie->bijd", W, x, optimize=True).astype(np.float32)


def _pack_w(W):
    """Build per-i-half wp (fp8 block-diag, x64) and wd (bf16 dense) arrays."""
    import ml_dtypes

    wps, wds = [], []
    for h in range(2):
        Wh = np.ascontiguousarray(W[h * IH : (h + 1) * IH])  # [576,10,16,8]
        Wb = Wh.astype(ml_dtypes.bfloat16)
        # [g, r, i3, jd, e]
        W5 = np.asarray(Wb, dtype=np.float32).reshape(NG, 4, IPG, JD, IN_DIM)
        # wd rows: [r, i3slot(4), e, g, jd]
        wd = np.zeros((4, 4, IN_DIM, NG, JD), dtype=ml_dtypes.bfloat16)
        wd[:, :IPG] = W5.transpose(1, 2, 4, 0, 3).astype(ml_dtypes.bfloat16)
        wds.append(np.ascontiguousarray(wd.reshape(128, NG, JD)))
        # wp: [r, i3slot, e, g, k, jd], nonzero at i3==k, scaled x64 in fp8
        Wq = (W5 * USCALE).astype(ml_dtypes.float8_e4m3)
        wp = np.zeros((4, 4, IN_DIM, NG, IPG, JD), dtype=ml_dtypes.float8_e4m3)
        Wqt = np.asarray(Wq, dtype=np.float32).transpose(1, 2, 4, 0, 3)  # [r,i3,e,g,jd]
        for k in range(IPG):
            wp[:, k, :, :, k, :] = Wqt[:, k].astype(ml_dtypes.float8_e4m3)
        wps.append(np.ascontiguousarray(wp.reshape(128, NG, FW)))
    return wps, wds


def _pack_x(x, q, h):
    import ml_dtypes

    xc = x[q * BC : (q + 1) * BC, h * IH : (h + 1) * IH]  # [128, 576, 8]
    x5 = xc.reshape(BC, NG, 4, IPG, IN_DIM)
    xt = np.zeros((4, 4, IN_DIM, NG, BC), dtype=ml_dtypes.bfloat16)
    xt[:, :IPG] = x5.transpose(2, 3, 4, 1, 0).astype(ml_dtypes.bfloat16)
    return np.ascontiguousarray(xt.reshape(128, NG, BC))


def kernel(x, W):
    x = np.asarray(x, dtype=np.float32)
    W = np.asarray(W, dtype=np.float32)
    try:
        from concourse.bass_utils import run_bass_kernel_spmd

        _install_ntff_hook()
        if "nc" not in _cached:
            _cached["nc"] = _build_nc()
        nc = _cached["nc"]
        wps, wds = _pack_w(W)
        in_maps = []
        for c in range(N_CORES):
            q, h = divmod(c, 2)
            in_maps.append({"xt": _pack_x(x, q, h), "wp": wps[h], "wd": wds[h]})
        try:
            res = run_bass_kernel_spmd(
                nc, in_maps, core_ids=list(range(N_CORES)), trace=True
            )
        except Exception:
            import traceback

            traceback.print_exc()
            res = run_bass_kernel_spmd(nc, in_maps, core_ids=list(range(N_CORES)))
        us = []
        s0s = []
        for c in range(N_CORES):
            u8 = np.asarray(res.results[c]["u8"], dtype=np.float32) / USCALE
            # [g, b, r, k, jd] -> [b, i_local, j, d]
            uc = u8.reshape(NG, BC, 4, IPG, JD).transpose(1, 0, 2, 3, 4)
            us.append(uc.reshape(BC, IH, OUT_CAPS, OUT_DIM))
            s0s.append(np.asarray(res.results[c]["s0"], dtype=np.float32))
        u = np.concatenate(
            [np.concatenate([us[2 * q], us[2 * q + 1]], axis=1) for q in range(NB)],
            axis=0,
        )
        S0 = np.concatenate(
            [(s0s[2 * q] + s0s[2 * q + 1]).reshape(BC, OUT_CAPS, OUT_DIM) for q in range(NB)],
            axis=0,
        )
        _cached["exec_time_ns"] = getattr(res, "exec_time_ns", None)
        return _routing_s0(u, S0)
    except Exception:
        import traceback

        traceback.print_exc()
        u = _u_host(x, W)
        return _routing(u)
